# revision 65
# baseline (speedup 1.0000x reference)
import sys

for _p in ("/opt/trn_rl_repo", "/opt/trn_rl_repo/concourse"):
    if _p not in sys.path:
        sys.path.insert(0, _p)

import numpy as np
import ml_dtypes

N_CORES = 8
B, H, W_DIM, C = 8, 32, 32, 288
NP = H * W_DIM         # 1024 points per image
N = 2 * NP             # 2048 points per core (one image PAIR per core)
O = 64                 # codewords total
OL = 32                # codewords per core (o-half sharding)
CHUNK = 512            # PSUM bank free size (fp32)
NCH = N // CHUNK       # 4 chunks
# Core c handles image pair (2p, 2p+1), p = c % 4, and codeword half
# h = c // 4 (o in [32h, 32h+32)). Doubling the free dim to 2048 amortizes
# the fixed per-op overheads (~280ns Act, ~250ns DVE pair) over 2x columns.

# Per-engine cost (us) of one [<=128, 2048] production op. Act computes
# |x-w| via Abs+bias (HW-measured ~1897ns issue-to-issue). DVE computes
# relu(x-w) via ONE fused tensor_scalar (x sub w) max 0, either -> fp8 at
# 2x_2p (~1240ns) feeding DoubleRow PE matmuls, or -> bf16 at 4x_2p
# (~674ns) feeding plain bf16 PE matmuls (2x PE cols, PE has slack);
# sum|d| = 2*sum relu(d) - sum x + sum w, with the -sum x term via extra
# PE matmuls over xa/xb/xt and +sum w baked into the bias on host.
# GPSIMD excluded (software TensorScalar ~15us/op).
COST_ACT = 1.897
COST_DVE8 = 1.240
COST_DVE16 = 0.674
# op counts per type (ops = 2*units): balances Act ~45.5us, DVE ~47.1us,
# PE ~44us, under the SBUF cap (bf16 unit slabs are 1MB vs fp8's 0.5MB;
# buffer rotation is off the table — a rotated producer op needs a WAR wait
# plus something else walrus cannot merge, overflowing the 1-wait slot).
N_DVE16_UNITS = 11

_CACHE = {}


def _patch_drain_split():
    # The end-of-TileContext drain waits on the FULL global clock (engines +
    # one sem per DMA HW queue), overflowing the CTRL_NO struct's sync-wait
    # slots in walrus. Split: emit one 1-wait SP nop per clock component
    # first; the original drain's full-clock add_sem_waits then elides
    # everything via SP wait history.
    import concourse.tile as tile_mod
    from concourse.vector_clock import ScopedClock, VectorClock

    if getattr(tile_mod.TileContext, "_drain_split_patched", False):
        return

    def _drain_and_barrier(self, tick_clock, wait_clock):
        gc = tick_clock.global_clock
        for idx in range(len(gc)):
            tick = gc[idx]
            if tick <= 0:
                continue
            nop = self.nc.sync.nop(nofuse=True, hint="drain_split")
            vc = VectorClock()
            vc.require_at_least(idx, tick)
            wait_clock.add_sem_waits(nop.ins, ScopedClock({None: vc}))
        # Waitless drain: the nops above (same SP sequencer, in order)
        # already guarantee every sem is at its final value here.
        self.nc.sync.drain()
        self.nc.all_engine_barrier()
        assert self.sems is not None
        popped = self.nc._tile_sem_poison_stack.pop()
        assert popped is self._sem_poison
        self.nc.clear_and_free_semaphores(list(self.sems.allocated().values()))
        self.nc.all_engine_barrier()

    tile_mod.TileContext._drain_and_barrier = _drain_and_barrier
    tile_mod.TileContext._drain_split_patched = True


def _assign_units():
    """Static engine assignment for the 36 production units (32 full-o units
    + 4 tail-pair units), each two [128, 2048] relu/abs ops. Greedily
    balances the Act vs DVE stream finish times; the DVE stream spends its
    first N_DVE16_UNITS picks as cheap bf16 units (4x mode), the rest fp8.
    Tail units are pinned to dve16 so no fp8 tail-routing tensor (and its
    DMA + PE absorber) is needed at all."""
    units = ([("full", o) for o in range(8)]
             + [("tail", gp) for gp in range(4)]
             + [("full", o) for o in range(8, OL)])
    t = {"act": 0.3, "dve": 0.3}
    dve16_left = [N_DVE16_UNITS - 4]
    out = []
    for u in units:
        if u[0] == "tail":
            t["dve"] += 2 * COST_DVE16
            out.append((u, "dve16"))
            continue
        # interleave bf16 units through the dve stream (every other pick)
        dve_kind = "dve16" if dve16_left[0] > 0 and (len(out) % 2 == 0) else "dve8"
        dve_cost = 2 * (COST_DVE16 if dve_kind == "dve16" else COST_DVE8)
        if t["act"] + 2 * COST_ACT <= t["dve"] + dve_cost:
            t["act"] += 2 * COST_ACT
            out.append((u, "act"))
        else:
            t["dve"] += dve_cost
            if dve_kind == "dve16":
                dve16_left[0] -= 1
            out.append((u, dve_kind))
    return out


ASSIGNMENT = _assign_units()
FULL_DVE = {a for (kind, a), e in ASSIGNMENT if kind == "full" and e != "act"}
TAIL_DVE = {a for (kind, a), e in ASSIGNMENT if kind == "tail" and e != "act"}


def _build_program():
    import concourse.bass as bass
    import concourse.tile as tile
    from concourse import mybir

    _patch_drain_split()
    nc = bass.Bass("TRN2", debug=False, num_devices=N_CORES)

    f32 = mybir.dt.float32
    f16 = mybir.dt.float16
    bf16 = mybir.dt.bfloat16
    fp8 = mybir.dt.float8e4
    Abs = mybir.ActivationFunctionType.Abs
    Ident = mybir.ActivationFunctionType.Identity
    DR = mybir.MatmulPerfMode.DoubleRow
    AOP = mybir.AluOpType

    # x transposed per core (2 images side by side): rows = channel, cols =
    # point. xa/xb are channel blocks 0:128 / 128:256; xt is channels
    # 256:288 replicated to all four SBUF quadrants so one op covers the
    # channel tail of four codewords.
    xa_d = nc.dram_tensor("xa", [128, N], bf16, kind="ExternalInput")
    xb_d = nc.dram_tensor("xb", [128, N], bf16, kind="ExternalInput")
    xt_d = nc.dram_tensor("xt", [128, N], bf16, kind="ExternalInput")
    # wcst cols (per o-half): 0:64 = -w for c-blocks (i*32+o), 64:72 =
    # quadrant-packed tail -w[256+j, 4g+q] at [32q+j, 64+g], 72:144 = +w
    # (same layout), col 144 = bias b (rows 0:32) + per-codeword sum-w
    # corrections for DVE-produced ranges.
    WC = 2 * OL + 8
    wcst_d = nc.dram_tensor("wcst", [128, 2 * WC + 1], f32, kind="ExternalInput")
    # bf16 constants blob (one DMA; >8 transfers would overflow the DMA
    # queues' single sync-wait slot): cols 0:128/128:256 = +2 tail routing
    # one-hots [32q+j, 32*gp + 4*(2gp+i)+q] for k-sub i=0/1 (tail units are
    # all dve16); cols 256:288 = -1 at full-DVE codewords (all rows); cols
    # 288:320 = -1 at tail codewords (rows 0:32); cols 320:352 row 0 = bias
    # (b + sum-w corrections), applied via a rank-1 matmul so extraction is
    # a pure PSUM copy.
    zmix_d = nc.dram_tensor("zmix", [128, 11 * OL], bf16, kind="ExternalInput")
    out_d = nc.dram_tensor("out_t", [OL, N], f16, kind="ExternalOutput")

    xa, xb, xt = xa_d.ap(), xb_d.ap(), xt_d.ap()
    wcst, out_t = wcst_d.ap(), out_d.ap()
    zmix_a = zmix_d.ap()

    from contextlib import ExitStack

    with tile.TileContext(nc) as tc, ExitStack() as ctx:
        const_pool = ctx.enter_context(tc.tile_pool(name="const", bufs=1))
        # Separate pools per (engine, slab dtype): fp8 pools rotate 6
        # buffers each. Rotation is PER ENGINE so the WAW dep on the
        # previous occupant is same-engine program order (no sem); the WAR
        # wait on the previous occupant's consuming matmuls then fits
        # walrus's single sync-wait slot (all DMA components are
        # pre-absorbed). bf16 slabs stay 1:1.
        n16 = sum(1 for _, e in ASSIGNMENT if e == "dve16")
        n_act = sum(1 for _, e in ASSIGNMENT if e == "act")
        prod_pool_act = ctx.enter_context(tc.tile_pool(name="prodA", bufs=n_act))
        prod_pool_d8 = ctx.enter_context(
            tc.tile_pool(name="prodD", bufs=36 - n16 - n_act))
        prod_pool16 = ctx.enter_context(tc.tile_pool(name="prod16", bufs=n16))
        psum_pool = ctx.enter_context(tc.tile_pool(name="ps", bufs=1, space="PSUM"))

        # --- SBUF constants. xa gates the first production ops, so it goes
        # first; wcst (needed by the same first ops via absorbers) is small
        # and lands within xa's transfer window.
        xa_sb = const_pool.tile([128, N], bf16, name="xa_sb")
        nc.sync.dma_start(xa_sb[:], xa[:, :])
        wcst_sb = const_pool.tile([128, 2 * WC + 1], f32, name="wcst_sb")
        nc.sync.dma_start(wcst_sb[:], wcst[:, :])
        xb_sb = const_pool.tile([128, N], bf16, name="xb_sb")
        nc.sync.dma_start(xb_sb[:], xb[:, :])
        xt_sb = const_pool.tile([128, N], bf16, name="xt_sb")
        nc.sync.dma_start(xt_sb[:], xt[:, :])
        zmix_sb = const_pool.tile([128, 11 * OL], bf16, name="zmix_sb")
        nc.sync.dma_start(zmix_sb[:], zmix_a[:, :])
        zcorr_sb = zmix_sb[:, 8 * OL : 11 * OL]
        negw_sb = wcst_sb[:, 0:WC]
        wbf_sb = wcst_sb[:, WC : 2 * WC]

        # Full-pair routing: constant column at absolute col 31 (both k-subs);
        # lhsT slice [:, :, 31-o : 63-o] puts the hot column at local index o.
        # zdr = +1 (act abs slabs), zdr2 = +2 (dve relu slabs).
        zdr = const_pool.tile([128, 2, 64], fp8, name="zdr")
        nc.vector.memset(zdr[:], 0.0)
        nc.vector.memset(zdr[:, :, 31:32], 1.0)
        zdr2 = const_pool.tile([128, 2, 64], fp8, name="zdr2")
        nc.vector.memset(zdr2[:], 0.0)
        nc.vector.memset(zdr2[:, :, 31:32], 2.0)
        # all-ones row for the rank-1 bias matmul
        ones_sb = const_pool.tile([1, CHUNK], bf16, name="ones_sb")
        nc.vector.memset(ones_sb[:], 1.0)
        # bf16 +2 routing for dve16 full units' plain (non-DR) matmuls
        zdr16 = const_pool.tile([128, 64], bf16, name="zdr16")
        nc.vector.memset(zdr16[:], 0.0)
        nc.vector.memset(zdr16[:, 31:32], 2.0)

        # --- walrus 1-sync-wait discipline: per-engine absorber ops ---------
        scr_d = const_pool.tile([1, 16], f32, name="scr_d")
        scr_a = const_pool.tile([1, 16], f32, name="scr_a")

        # Each engine's production ops write fresh buffers (no WAR), so a
        # single sem wait per op suffices IF all DMA components it reads are
        # pre-absorbed into the engine's wait history. Pre-absorb every
        # tensor each stream reads. xa comes in two half-row DMAs, so its
        # first half needs its own absorber on BOTH streams (the first
        # production op then carries only the second half's sem).
        for k, s in enumerate((wcst_sb, xa_sb, xb_sb, xt_sb)):
            nc.vector.tensor_scalar_add(scr_d[0:1, k : k + 1], s[0:1, 0:1], 0.0)
        nc.scalar.activation(scr_a[0:1, 0:1], wcst_sb[0:1, 0:1], Abs,
                             bias=wcst_sb[0:1, 0:1])

        # --- PSUM banks: one accumulation group per 512-col chunk -----------
        bank = [psum_pool.tile([128, CHUNK], f32, name=f"bank{ch}")
                for ch in range(NCH)]
        tinyb = psum_pool.tile([128, CHUNK], f32, name="tinyb")

        # PE absorber: load the DVE memset sems into PE wait history via a
        # singleton matmul before the real stream (each instruction may
        # carry only ONE sync wait). zdr16 is the LAST memset in the DVE
        # stream, so waiting on it covers all. The zmix-DMA absorber is
        # emitted later, just before the first tail unit's matmuls — that
        # way PE starts on unit 0 as soon as its slab is ready instead of
        # stalling on the (late-landing) constants DMA.
        nc.tensor.matmul(tinyb[0:1, 0:1], lhsT=zdr16[:, 0:1], rhs=zdr16[:, 0:1],
                         start=True, stop=True)

        def produce(eng, dst, src, col):
            if eng == "act":
                nc.scalar.activation(dst, src, Abs,
                                     bias=negw_sb[:, col : col + 1])
            else:
                # fused relu(x - w) in ONE DVE pass: (x sub w) max 0
                nc.vector.tensor_scalar(dst, src, wbf_sb[:, col : col + 1],
                                        0.0, op0=AOP.subtract, op1=AOP.max)

        assignment = ASSIGNMENT
        n_units = len(assignment)
        started = [False] * NCH  # per-bank accumulation-group start tracking

        def corrections():
            # -sum(x) corrections for DVE relu units (accumulate
            # -sum_c x[c, n] into each DVE-produced codeword row) plus the
            # rank-1 bias add. Emitted mid-stream (after unit 7): PE idles
            # between units there (it drains a unit's matmuls faster than
            # the engines produce slabs), and all inputs landed long ago.
            for src in (xa_sb, xb_sb):
                for ch in range(NCH):
                    nc.tensor.matmul(
                        bank[ch][0:OL, :],
                        lhsT=zcorr_sb[:, 0:OL],
                        rhs=src[:, CHUNK * ch : CHUNK * (ch + 1)],
                        start=False, stop=False,
                    )
            for ch in range(NCH):
                nc.tensor.matmul(
                    bank[ch][0:OL, :],
                    lhsT=zcorr_sb[0:32, OL : 2 * OL],
                    rhs=xt_sb[0:32, CHUNK * ch : CHUNK * (ch + 1)],
                    start=False, stop=False,
                )
            for ch in range(NCH):
                nc.tensor.matmul(
                    bank[ch][0:OL, :],
                    lhsT=zcorr_sb[0:1, 2 * OL : 3 * OL],
                    rhs=ones_sb[0:1, :],
                    start=False, stop=False,
                )

        for ui, ((kind, a), eng) in enumerate(assignment):
            last = ui == n_units - 1
            if ui == 8:
                # first tail unit is next: absorb the zmix and xa-half-1 DMA
                # sems now (they have long since landed — no stall), then run
                # corrections in PE idle time.
                nc.tensor.matmul(tinyb[0:1, 0:1], lhsT=zmix_sb[:, 0:1],
                                 rhs=zmix_sb[:, 0:1], start=True, stop=True)
                corrections()
            if eng == "dve16":
                dt = prod_pool16.tile([128, 2, N], bf16, name="dt", tag="u16")
            elif eng == "act":
                dt = prod_pool_act.tile([128, 2, N], fp8, name="dt", tag="uA")
            else:
                dt = prod_pool_d8.tile([128, 2, N], fp8, name="dt", tag="uD")
            if kind == "full":
                o = a
                for i, src in enumerate((xa_sb, xb_sb)):
                    produce(eng, dt[:, i, :], src, i * OL + o)
            else:
                gp = a
                for i in range(2):
                    produce(eng, dt[:, i, :], xt_sb, 2 * OL + 2 * gp + i)
            if eng == "dve16":
                # plain bf16 matmuls: one per (k-sub, chunk)
                if kind == "full":
                    lhsT_i = [zdr16[:, 31 - a : 63 - a]] * 2
                else:
                    lhsT_i = [zmix_sb[:, 4 * OL * i + OL * a : 4 * OL * i + OL * (a + 1)]
                              for i in range(2)]
                for ch in range(NCH):
                    for i in range(2):
                        nc.tensor.matmul(
                            bank[ch][0:OL, :],
                            lhsT=lhsT_i[i],
                            rhs=dt[:, i, CHUNK * ch : CHUNK * (ch + 1)],
                            start=(not started[ch] and i == 0),
                            stop=(last and i == 1),
                        )
                    started[ch] = True
            else:
                assert kind == "full"
                zroute = zdr if eng == "act" else zdr2
                lhsT = zroute[:, :, 31 - a : 63 - a]
                for ch in range(NCH):
                    nc.tensor.matmul(
                        bank[ch][0:OL, :],
                        lhsT=lhsT,
                        rhs=dt[:, :, CHUNK * ch : CHUNK * (ch + 1)],
                        start=not started[ch],
                        stop=last,
                        perf_mode=DR,
                    )
                    started[ch] = True

        # --- output: bias is already accumulated in PSUM (rank-1 matmul),
        # so extraction is a pure PSUM->SBUF copy (fp16 staging to halve its
        # SBUF footprint); chunks 0,1 on DVE and 2,3 on Act in parallel.
        out_sb = const_pool.tile([OL, N], f16, name="out_sb")
        for ch in (0, 1):
            nc.vector.tensor_scalar_add(
                out_sb[0:OL, CHUNK * ch : CHUNK * (ch + 1)],
                bank[ch][0:OL, :], 0.0)
        nc.sync.dma_start(out_t[:, 0 : 2 * CHUNK], out_sb[0:OL, 0 : 2 * CHUNK])
        for ch in (2, 3):
            nc.scalar.activation(
                out_sb[0:OL, CHUNK * ch : CHUNK * (ch + 1)],
                bank[ch][0:OL, :], Ident)
        nc.sync.dma_start(out_t[:, 2 * CHUNK : 4 * CHUNK],
                          out_sb[0:OL, 2 * CHUNK : 4 * CHUNK])

    return nc


def _prep_inputs(x, w, b):
    xs = x.reshape(B, NP, C).astype(np.float32)
    w = np.asarray(w, dtype=np.float32)
    b = np.asarray(b, dtype=np.float32)
    fp8 = ml_dtypes.float8_e4m3
    bf16 = ml_dtypes.bfloat16
    WC = 2 * OL + 8

    wcsts = []
    for h in range(2):
        wh = w[:, OL * h : OL * (h + 1)]  # [288, 32]
        negw = np.zeros((128, WC), dtype=np.float32)
        for i in range(2):
            negw[:, i * OL : (i + 1) * OL] = -wh[128 * i : 128 * (i + 1), :]
        for g in range(8):
            for q in range(4):
                negw[32 * q : 32 * q + 32, 2 * OL + g] = -wh[256:288, 4 * g + q]
        wcst = np.zeros((128, 2 * WC + 1), dtype=np.float32)
        wcst[:, 0:WC] = negw
        wcst[:, WC : 2 * WC] = -negw
        # bias + the +sum(w) part of sum|d| = 2*sum relu(d) - sum x + sum w
        # for DVE-produced (codeword, channel-range) pieces
        bias = b[OL * h : OL * (h + 1)].copy()
        for o in range(OL):
            if o in FULL_DVE:
                bias[o] += wh[0:256, o].sum()
            if (o // 8) in TAIL_DVE:
                bias[o] += wh[256:288, o].sum()
        wcst[0:OL, 2 * WC] = bias
        wcsts.append(wcst)

    ztail16 = np.zeros((128, 2, 4 * OL), dtype=np.float32)
    for gp in range(4):
        for i in range(2):
            for q in range(4):
                o = 4 * (2 * gp + i) + q
                ztail16[32 * q : 32 * q + 32, i, OL * gp + o] = 2.0

    zmixes = []
    for h in range(2):
        zmix = np.zeros((128, 11 * OL), dtype=np.float32)
        zmix[:, 0 : 4 * OL] = ztail16[:, 0, :]
        zmix[:, 4 * OL : 8 * OL] = ztail16[:, 1, :]
        for o in FULL_DVE:
            zmix[:, 8 * OL + o] = -1.0
        for gp in TAIL_DVE:
            for o in range(8 * gp, 8 * gp + 8):
                zmix[0:32, 9 * OL + o] = -1.0
        zmix[0, 10 * OL : 11 * OL] = wcsts[h][0:OL, 2 * WC]  # bias row
        zmixes.append(zmix.astype(bf16))

    in_maps = []
    for core in range(N_CORES):
        p, h = core % 4, core // 4
        xT = np.concatenate([xs[2 * p].T, xs[2 * p + 1].T], axis=1)  # [C, 2048]
        in_maps.append({
            "xa": xT[0:128].astype(bf16),
            "xb": xT[128:256].astype(bf16),
            "xt": np.tile(xT[256:288], (4, 1)).astype(bf16),
            "wcst": wcsts[h], "zmix": zmixes[h],
        })
    return in_maps


def kernel(x, w, b):
    from concourse.bass_utils import run_bass_kernel_spmd

    if "nc" not in _CACHE:
        _CACHE["nc"] = _build_program()
    nc = _CACHE["nc"]

    in_maps = _prep_inputs(x, w, b)
    res = run_bass_kernel_spmd(nc, in_maps, list(range(N_CORES)))
    out = np.empty((B, NP, O), dtype=np.float32)
    for core in range(N_CORES):
        p, h = core % 4, core // 4
        r = np.asarray(res.results[core]["out_t"], dtype=np.float32)  # [OL, 2048]
        out[2 * p, :, OL * h : OL * (h + 1)] = r[:, 0:NP].T
        out[2 * p + 1, :, OL * h : OL * (h + 1)] = r[:, NP : 2 * NP].T
    return out



# revision 66
# speedup vs baseline: 1.0067x; 1.0067x over previous
import sys

for _p in ("/opt/trn_rl_repo", "/opt/trn_rl_repo/concourse"):
    if _p not in sys.path:
        sys.path.insert(0, _p)

import numpy as np
import ml_dtypes

N_CORES = 8
B, H, W_DIM, C = 8, 32, 32, 288
NP = H * W_DIM         # 1024 points per image
N = 2 * NP             # 2048 points per core (one image PAIR per core)
O = 64                 # codewords total
OL = 32                # codewords per core (o-half sharding)
CHUNK = 512            # PSUM bank free size (fp32)
NCH = N // CHUNK       # 4 chunks
# Core c handles image pair (2p, 2p+1), p = c % 4, and codeword half
# h = c // 4 (o in [32h, 32h+32)). Doubling the free dim to 2048 amortizes
# the fixed per-op overheads (~280ns Act, ~250ns DVE pair) over 2x columns.

# Per-engine cost (us) of one [<=128, 2048] production op. Act computes
# |x-w| via Abs+bias (HW-measured ~1897ns issue-to-issue). DVE computes
# relu(x-w) via ONE fused tensor_scalar (x sub w) max 0, either -> fp8 at
# 2x_2p (~1240ns) feeding DoubleRow PE matmuls, or -> bf16 at 4x_2p
# (~674ns) feeding plain bf16 PE matmuls (2x PE cols, PE has slack);
# sum|d| = 2*sum relu(d) - sum x + sum w, with the -sum x term via extra
# PE matmuls over xa/xb/xt and +sum w baked into the bias on host.
# GPSIMD excluded (software TensorScalar ~15us/op).
COST_ACT = 1.897
COST_DVE8 = 1.240
COST_DVE16 = 0.674
# op counts per type (ops = 2*units): balances Act ~45.5us, DVE ~47.1us,
# PE ~44us, under the SBUF cap (bf16 unit slabs are 1MB vs fp8's 0.5MB;
# buffer rotation is off the table — a rotated producer op needs a WAR wait
# plus something else walrus cannot merge, overflowing the 1-wait slot).
N_DVE16_UNITS = 11

_CACHE = {}


def _patch_drain_split():
    # The end-of-TileContext drain waits on the FULL global clock (engines +
    # one sem per DMA HW queue), overflowing the CTRL_NO struct's sync-wait
    # slots in walrus. Split: emit one 1-wait SP nop per clock component
    # first; the original drain's full-clock add_sem_waits then elides
    # everything via SP wait history.
    import concourse.tile as tile_mod
    from concourse.vector_clock import ScopedClock, VectorClock

    if getattr(tile_mod.TileContext, "_drain_split_patched", False):
        return

    def _drain_and_barrier(self, tick_clock, wait_clock):
        gc = tick_clock.global_clock
        for idx in range(len(gc)):
            tick = gc[idx]
            if tick <= 0:
                continue
            nop = self.nc.sync.nop(nofuse=True, hint="drain_split")
            vc = VectorClock()
            vc.require_at_least(idx, tick)
            wait_clock.add_sem_waits(nop.ins, ScopedClock({None: vc}))
        # Waitless drain: the nops above (same SP sequencer, in order)
        # already guarantee every sem is at its final value here.
        self.nc.sync.drain()
        self.nc.all_engine_barrier()
        assert self.sems is not None
        popped = self.nc._tile_sem_poison_stack.pop()
        assert popped is self._sem_poison
        self.nc.clear_and_free_semaphores(list(self.sems.allocated().values()))
        self.nc.all_engine_barrier()

    tile_mod.TileContext._drain_and_barrier = _drain_and_barrier
    tile_mod.TileContext._drain_split_patched = True


def _assign_units():
    """Static engine assignment for the 36 production units (32 full-o units
    + 4 tail-pair units), each two [128, 2048] relu/abs ops. Greedily
    balances the Act vs DVE stream finish times; the DVE stream spends its
    first N_DVE16_UNITS picks as cheap bf16 units (4x mode), the rest fp8.
    Tail units are pinned to dve16 so no fp8 tail-routing tensor (and its
    DMA + PE absorber) is needed at all."""
    units = ([("full", o) for o in range(8)]
             + [("tail", gp) for gp in range(4)]
             + [("full", o) for o in range(8, OL)])
    t = {"act": 0.3, "dve": 0.3}
    dve16_left = [N_DVE16_UNITS - 4]
    out = []
    for u in units:
        if u[0] == "tail":
            t["dve"] += 2 * COST_DVE16
            out.append((u, "dve16"))
            continue
        # interleave bf16 units through the dve stream (every other pick)
        dve_kind = "dve16" if dve16_left[0] > 0 and (len(out) % 2 == 0) else "dve8"
        dve_cost = 2 * (COST_DVE16 if dve_kind == "dve16" else COST_DVE8)
        if t["act"] + 2 * COST_ACT <= t["dve"] + dve_cost:
            t["act"] += 2 * COST_ACT
            out.append((u, "act"))
        else:
            t["dve"] += dve_cost
            if dve_kind == "dve16":
                dve16_left[0] -= 1
            out.append((u, dve_kind))
    return out


ASSIGNMENT = _assign_units()
FULL_DVE = {a for (kind, a), e in ASSIGNMENT if kind == "full" and e != "act"}
TAIL_DVE = {a for (kind, a), e in ASSIGNMENT if kind == "tail" and e != "act"}


def _build_program():
    import concourse.bass as bass
    import concourse.tile as tile
    from concourse import mybir

    _patch_drain_split()
    nc = bass.Bass("TRN2", debug=False, num_devices=N_CORES)

    f32 = mybir.dt.float32
    f16 = mybir.dt.float16
    bf16 = mybir.dt.bfloat16
    fp8 = mybir.dt.float8e4
    Abs = mybir.ActivationFunctionType.Abs
    Ident = mybir.ActivationFunctionType.Identity
    DR = mybir.MatmulPerfMode.DoubleRow
    AOP = mybir.AluOpType

    # x transposed per core (2 images side by side): rows = channel, cols =
    # point. xa/xb are channel blocks 0:128 / 128:256; xt is channels
    # 256:288 replicated to all four SBUF quadrants so one op covers the
    # channel tail of four codewords.
    xa_d = nc.dram_tensor("xa", [128, N], bf16, kind="ExternalInput")
    xb_d = nc.dram_tensor("xb", [128, N], bf16, kind="ExternalInput")
    xt_d = nc.dram_tensor("xt", [128, N], bf16, kind="ExternalInput")
    # wcst cols (per o-half): 0:64 = -w for c-blocks (i*32+o), 64:72 =
    # quadrant-packed tail -w[256+j, 4g+q] at [32q+j, 64+g], 72:144 = +w
    # (same layout), col 144 = bias b (rows 0:32) + per-codeword sum-w
    # corrections for DVE-produced ranges.
    WC = 2 * OL + 8
    wcst_d = nc.dram_tensor("wcst", [128, 2 * WC + 1], f32, kind="ExternalInput")
    # bf16 constants blob (one DMA; >8 transfers would overflow the DMA
    # queues' single sync-wait slot): cols 0:128/128:256 = +2 tail routing
    # one-hots [32q+j, 32*gp + 4*(2gp+i)+q] for k-sub i=0/1 (tail units are
    # all dve16); cols 256:288 = -1 at full-DVE codewords (all rows); cols
    # 288:320 = -1 at tail codewords (rows 0:32); cols 320:352 row 0 = bias
    # (b + sum-w corrections), applied via a rank-1 matmul so extraction is
    # a pure PSUM copy.
    zmix_d = nc.dram_tensor("zmix", [128, 11 * OL], bf16, kind="ExternalInput")
    out_d = nc.dram_tensor("out_t", [OL, N], f16, kind="ExternalOutput")

    xa, xb, xt = xa_d.ap(), xb_d.ap(), xt_d.ap()
    wcst, out_t = wcst_d.ap(), out_d.ap()
    zmix_a = zmix_d.ap()

    from contextlib import ExitStack

    with tile.TileContext(nc) as tc, ExitStack() as ctx:
        const_pool = ctx.enter_context(tc.tile_pool(name="const", bufs=1))
        # Separate pools per (engine, slab dtype): fp8 pools rotate 6
        # buffers each. Rotation is PER ENGINE so the WAW dep on the
        # previous occupant is same-engine program order (no sem); the WAR
        # wait on the previous occupant's consuming matmuls then fits
        # walrus's single sync-wait slot (all DMA components are
        # pre-absorbed). bf16 slabs stay 1:1.
        n16 = sum(1 for _, e in ASSIGNMENT if e == "dve16")
        n_act = sum(1 for _, e in ASSIGNMENT if e == "act")
        prod_pool_act = ctx.enter_context(tc.tile_pool(name="prodA", bufs=n_act))
        prod_pool_d8 = ctx.enter_context(
            tc.tile_pool(name="prodD", bufs=36 - n16 - n_act))
        prod_pool16 = ctx.enter_context(tc.tile_pool(name="prod16", bufs=n16))
        psum_pool = ctx.enter_context(tc.tile_pool(name="ps", bufs=1, space="PSUM"))

        # --- SBUF constants. xa gates the first production ops, so it goes
        # first; wcst (needed by the same first ops via absorbers) is small
        # and lands within xa's transfer window.
        xa_sb = const_pool.tile([128, N], bf16, name="xa_sb")
        nc.sync.dma_start(xa_sb[:], xa[:, :])
        wcst_sb = const_pool.tile([128, 2 * WC + 1], f32, name="wcst_sb")
        nc.sync.dma_start(wcst_sb[:], wcst[:, :])
        xb_sb = const_pool.tile([128, N], bf16, name="xb_sb")
        nc.sync.dma_start(xb_sb[:], xb[:, :])
        xt_sb = const_pool.tile([128, N], bf16, name="xt_sb")
        nc.sync.dma_start(xt_sb[:], xt[:, :])
        zmix_sb = const_pool.tile([128, 11 * OL], bf16, name="zmix_sb")
        nc.sync.dma_start(zmix_sb[:], zmix_a[:, :])
        zcorr_sb = zmix_sb[:, 8 * OL : 11 * OL]
        negw_sb = wcst_sb[:, 0:WC]
        wbf_sb = wcst_sb[:, WC : 2 * WC]

        # Full-pair routing: constant column at absolute col 31 (both k-subs);
        # lhsT slice [:, :, 31-o : 63-o] puts the hot column at local index o.
        # zdr = +1 (act abs slabs), zdr2 = +2 (dve relu slabs).
        zdr = const_pool.tile([128, 2, 64], fp8, name="zdr")
        nc.vector.memset(zdr[:], 0.0)
        nc.vector.memset(zdr[:, :, 31:32], 1.0)
        zdr2 = const_pool.tile([128, 2, 64], fp8, name="zdr2")
        nc.vector.memset(zdr2[:], 0.0)
        nc.vector.memset(zdr2[:, :, 31:32], 2.0)
        # all-ones row for the rank-1 bias matmul
        ones_sb = const_pool.tile([1, CHUNK], bf16, name="ones_sb")
        nc.vector.memset(ones_sb[:], 1.0)
        # bf16 +2 routing for dve16 full units' plain (non-DR) matmuls
        zdr16 = const_pool.tile([128, 64], bf16, name="zdr16")
        nc.vector.memset(zdr16[:], 0.0)
        nc.vector.memset(zdr16[:, 31:32], 2.0)

        # --- walrus 1-sync-wait discipline: per-engine absorber ops ---------
        scr_d = const_pool.tile([1, 16], f32, name="scr_d")
        scr_a = const_pool.tile([1, 16], f32, name="scr_a")

        # Each engine's production ops write fresh buffers (no WAR), so a
        # single sem wait per op suffices IF all DMA components it reads are
        # pre-absorbed into the engine's wait history. Pre-absorb every
        # tensor each stream reads. xa comes in two half-row DMAs, so its
        # first half needs its own absorber on BOTH streams (the first
        # production op then carries only the second half's sem).
        for k, s in enumerate((wcst_sb, xa_sb, xb_sb, xt_sb)):
            nc.vector.tensor_scalar_add(scr_d[0:1, k : k + 1], s[0:1, 0:1], 0.0)
        nc.scalar.activation(scr_a[0:1, 0:1], wcst_sb[0:1, 0:1], Abs,
                             bias=wcst_sb[0:1, 0:1])

        # --- PSUM banks: one accumulation group per 512-col chunk -----------
        bank = [psum_pool.tile([128, CHUNK], f32, name=f"bank{ch}")
                for ch in range(NCH)]
        tinyb = psum_pool.tile([128, CHUNK], f32, name="tinyb")

        # PE absorber: load the DVE memset sems into PE wait history via a
        # singleton matmul before the real stream (each instruction may
        # carry only ONE sync wait). zdr16 is the LAST memset in the DVE
        # stream, so waiting on it covers all. The zmix-DMA absorber is
        # emitted later, just before the first tail unit's matmuls — that
        # way PE starts on unit 0 as soon as its slab is ready instead of
        # stalling on the (late-landing) constants DMA.
        nc.tensor.matmul(tinyb[0:1, 0:1], lhsT=zdr16[:, 0:1], rhs=zdr16[:, 0:1],
                         start=True, stop=True)

        def produce(eng, dst, src, col):
            if eng == "act":
                nc.scalar.activation(dst, src, Abs,
                                     bias=negw_sb[:, col : col + 1])
            else:
                # fused relu(x - w) in ONE DVE pass: (x sub w) max 0
                nc.vector.tensor_scalar(dst, src, wbf_sb[:, col : col + 1],
                                        0.0, op0=AOP.subtract, op1=AOP.max)

        assignment = ASSIGNMENT
        n_units = len(assignment)
        started = [False] * NCH  # per-bank accumulation-group start tracking

        def corrections():
            # -sum(x) corrections for DVE relu units (accumulate
            # -sum_c x[c, n] into each DVE-produced codeword row) plus the
            # rank-1 bias add. Emitted mid-stream (after unit 7): PE idles
            # between units there (it drains a unit's matmuls faster than
            # the engines produce slabs), and all inputs landed long ago.
            for src in (xa_sb, xb_sb):
                for ch in range(NCH):
                    nc.tensor.matmul(
                        bank[ch][0:OL, :],
                        lhsT=zcorr_sb[:, 0:OL],
                        rhs=src[:, CHUNK * ch : CHUNK * (ch + 1)],
                        start=False, stop=False,
                    )
            for ch in range(NCH):
                nc.tensor.matmul(
                    bank[ch][0:OL, :],
                    lhsT=zcorr_sb[0:32, OL : 2 * OL],
                    rhs=xt_sb[0:32, CHUNK * ch : CHUNK * (ch + 1)],
                    start=False, stop=False,
                )
            for ch in range(NCH):
                nc.tensor.matmul(
                    bank[ch][0:OL, :],
                    lhsT=zcorr_sb[0:1, 2 * OL : 3 * OL],
                    rhs=ones_sb[0:1, :],
                    start=False, stop=False,
                )

        # Pre-plan production ops per engine stream so the first two units
        # of each stream can interleave their ops ([u0.xa, u1.xa, u0.xb,
        # u1.xb]): xb lands ~2.5us after xa, and op-interleaving keeps both
        # engines busy on xa-data during that window instead of stalling.
        tiles = []
        plan = {"act": [], "dve": []}
        for ui, ((kind, a), eng) in enumerate(assignment):
            if eng == "dve16":
                dt = prod_pool16.tile([128, 2, N], bf16, name="dt", tag="u16")
            elif eng == "act":
                dt = prod_pool_act.tile([128, 2, N], fp8, name="dt", tag="uA")
            else:
                dt = prod_pool_d8.tile([128, 2, N], fp8, name="dt", tag="uD")
            tiles.append(dt)
            stream = "act" if eng == "act" else "dve"
            if kind == "full":
                for i, src in enumerate((xa_sb, xb_sb)):
                    plan[stream].append((eng, dt[:, i, :], src, i * OL + a))
            else:
                for i in range(2):
                    plan[stream].append(
                        (eng, dt[:, i, :], xt_sb, 2 * OL + 2 * a + i))
        for stream in ("act", "dve"):
            ops = plan[stream]
            order = [0, 2, 1, 3] + list(range(4, len(ops)))
            for k in order:
                produce(*ops[k])

        for ui, ((kind, a), eng) in enumerate(assignment):
            last = ui == n_units - 1
            dt = tiles[ui]
            if ui == 8:
                # first tail unit is next: absorb the zmix DMA sem now (it
                # has long since landed — no stall), then run corrections in
                # PE idle time.
                nc.tensor.matmul(tinyb[0:1, 0:1], lhsT=zmix_sb[:, 0:1],
                                 rhs=zmix_sb[:, 0:1], start=True, stop=True)
                corrections()
            if eng == "dve16":
                # plain bf16 matmuls: one per (k-sub, chunk)
                if kind == "full":
                    lhsT_i = [zdr16[:, 31 - a : 63 - a]] * 2
                else:
                    lhsT_i = [zmix_sb[:, 4 * OL * i + OL * a : 4 * OL * i + OL * (a + 1)]
                              for i in range(2)]
                for ch in range(NCH):
                    for i in range(2):
                        nc.tensor.matmul(
                            bank[ch][0:OL, :],
                            lhsT=lhsT_i[i],
                            rhs=dt[:, i, CHUNK * ch : CHUNK * (ch + 1)],
                            start=(not started[ch] and i == 0),
                            stop=(last and i == 1),
                        )
                    started[ch] = True
            else:
                assert kind == "full"
                zroute = zdr if eng == "act" else zdr2
                lhsT = zroute[:, :, 31 - a : 63 - a]
                for ch in range(NCH):
                    nc.tensor.matmul(
                        bank[ch][0:OL, :],
                        lhsT=lhsT,
                        rhs=dt[:, :, CHUNK * ch : CHUNK * (ch + 1)],
                        start=not started[ch],
                        stop=last,
                        perf_mode=DR,
                    )
                    started[ch] = True

        # --- output: bias is already accumulated in PSUM (rank-1 matmul),
        # so extraction is a pure PSUM->SBUF copy (fp16 staging to halve its
        # SBUF footprint); chunks 0,1 on DVE and 2,3 on Act in parallel.
        out_sb = const_pool.tile([OL, N], f16, name="out_sb")
        for ch in (0, 1):
            nc.vector.tensor_scalar_add(
                out_sb[0:OL, CHUNK * ch : CHUNK * (ch + 1)],
                bank[ch][0:OL, :], 0.0)
        nc.sync.dma_start(out_t[:, 0 : 2 * CHUNK], out_sb[0:OL, 0 : 2 * CHUNK])
        for ch in (2, 3):
            nc.scalar.activation(
                out_sb[0:OL, CHUNK * ch : CHUNK * (ch + 1)],
                bank[ch][0:OL, :], Ident)
        nc.sync.dma_start(out_t[:, 2 * CHUNK : 4 * CHUNK],
                          out_sb[0:OL, 2 * CHUNK : 4 * CHUNK])

    return nc


def _prep_inputs(x, w, b):
    xs = x.reshape(B, NP, C).astype(np.float32)
    w = np.asarray(w, dtype=np.float32)
    b = np.asarray(b, dtype=np.float32)
    fp8 = ml_dtypes.float8_e4m3
    bf16 = ml_dtypes.bfloat16
    WC = 2 * OL + 8

    wcsts = []
    for h in range(2):
        wh = w[:, OL * h : OL * (h + 1)]  # [288, 32]
        negw = np.zeros((128, WC), dtype=np.float32)
        for i in range(2):
            negw[:, i * OL : (i + 1) * OL] = -wh[128 * i : 128 * (i + 1), :]
        for g in range(8):
            for q in range(4):
                negw[32 * q : 32 * q + 32, 2 * OL + g] = -wh[256:288, 4 * g + q]
        wcst = np.zeros((128, 2 * WC + 1), dtype=np.float32)
        wcst[:, 0:WC] = negw
        wcst[:, WC : 2 * WC] = -negw
        # bias + the +sum(w) part of sum|d| = 2*sum relu(d) - sum x + sum w
        # for DVE-produced (codeword, channel-range) pieces
        bias = b[OL * h : OL * (h + 1)].copy()
        for o in range(OL):
            if o in FULL_DVE:
                bias[o] += wh[0:256, o].sum()
            if (o // 8) in TAIL_DVE:
                bias[o] += wh[256:288, o].sum()
        wcst[0:OL, 2 * WC] = bias
        wcsts.append(wcst)

    ztail16 = np.zeros((128, 2, 4 * OL), dtype=np.float32)
    for gp in range(4):
        for i in range(2):
            for q in range(4):
                o = 4 * (2 * gp + i) + q
                ztail16[32 * q : 32 * q + 32, i, OL * gp + o] = 2.0

    zmixes = []
    for h in range(2):
        zmix = np.zeros((128, 11 * OL), dtype=np.float32)
        zmix[:, 0 : 4 * OL] = ztail16[:, 0, :]
        zmix[:, 4 * OL : 8 * OL] = ztail16[:, 1, :]
        for o in FULL_DVE:
            zmix[:, 8 * OL + o] = -1.0
        for gp in TAIL_DVE:
            for o in range(8 * gp, 8 * gp + 8):
                zmix[0:32, 9 * OL + o] = -1.0
        zmix[0, 10 * OL : 11 * OL] = wcsts[h][0:OL, 2 * WC]  # bias row
        zmixes.append(zmix.astype(bf16))

    in_maps = []
    for core in range(N_CORES):
        p, h = core % 4, core // 4
        xT = np.concatenate([xs[2 * p].T, xs[2 * p + 1].T], axis=1)  # [C, 2048]
        in_maps.append({
            "xa": xT[0:128].astype(bf16),
            "xb": xT[128:256].astype(bf16),
            "xt": np.tile(xT[256:288], (4, 1)).astype(bf16),
            "wcst": wcsts[h], "zmix": zmixes[h],
        })
    return in_maps


def kernel(x, w, b):
    from concourse.bass_utils import run_bass_kernel_spmd

    if "nc" not in _CACHE:
        _CACHE["nc"] = _build_program()
    nc = _CACHE["nc"]

    in_maps = _prep_inputs(x, w, b)
    res = run_bass_kernel_spmd(nc, in_maps, list(range(N_CORES)))
    out = np.empty((B, NP, O), dtype=np.float32)
    for core in range(N_CORES):
        p, h = core % 4, core // 4
        r = np.asarray(res.results[core]["out_t"], dtype=np.float32)  # [OL, 2048]
        out[2 * p, :, OL * h : OL * (h + 1)] = r[:, 0:NP].T
        out[2 * p + 1, :, OL * h : OL * (h + 1)] = r[:, NP : 2 * NP].T
    return out

